# revision 51
# baseline (speedup 1.0000x reference)
"""Trainium2 Bass kernel for BLiqNet (liquid-ODE net), 8-core data parallel.

Math (per batch row):
    u  = x @ Wx.T + bx
    dh/dt = (-h + tanh(W h + U u + b)) / tau,  h(0) = u, t in [0, 1]
    y  = h(1) @ Wf.T + bf

Integrator: a single step of ETDRK3 (Cox-Matthews exponential RK3) over
dt = 1.  The linear part L = -1/tau is diagonal, so all phi-function
coefficients are per-hidden-unit vectors, precomputed on the host in
fp64.  Accuracy vs the 40-step RK4 reference: ~4.5e-3 relmax (fp16
device pipeline emulated; measured ETDRK4 variant matched its emulation
within 1e-4), inside the 2e-2 gate with 4x margin.

Device-side restructure ("u-fold"): the latent projection u never
materializes on device.  With stage states s_i, the PSUM-resident tensor
P always equals s_i @ W.T + u @ U.T:

    P1  = x @ M1.T                       M1 = (W+U) Wx          (K=256)
    t1  = tanh(P + bias1)
    P  += t1 @ Wg2.T + x @ M2.T          Wg2 = W diag(gam2)
                                         M2  = W diag(E2-1) Wx
    t2  = tanh(P + bias2)
    P  += (2 t2 - r t1) @ Wg1.T + x @ M3.T
                                         Wg1 = W diag(gam1)
                                         M3  = W diag(E-E2) Wx, r = (gam1+gam2)/gam1
    t3  = tanh(P + bias3)
    d   = g2 t2 + g3 t3                  (broadcast-constant tensor_tensor)
    yT  = Wf d + Wf1 t1 + Mh x + cy      Wf1 = Wf diag(g1), Mh = Wf diag(E) Wx

with E2 = exp(-1/(2 tau)), E = exp(-1/tau), gam2 = phi1(z/2)/(2 tau),
gam1 = phi1(z)/tau, g1..g3 the ETDRK3 output weights over dt=1, and all
per-eval constants folded into the tanh bias vectors.  Every matmul is
fp16 x fp16 with a [128,128] stationary tile and N=512 moving columns;
PSUM accumulates fp32.  Elementwise work is plain tensor_tensor on the
DVE (scalar_tensor_tensor and gpsimd measured 3-5x slower); per-unit
constants ship as host-precomputed broadcast tiles.

Layout: hidden 512 = 4 tiles x 128 partitions; batch 4096/core as 8
chunks of 512 columns, two in flight (P = 2 x 4 PSUM banks = all 8
banks, allocated from one long-lived pool with rotating tags so pass
boundaries only serialize per-chunk).  The head reuses chunk bank 0
after the last tanh read.  The head is computed transposed (partitions =
128 outputs, columns = batch) so the output DMA is layout-direct; the
host transposes once at the end.
"""
import numpy as np

import concourse.bass as bass
import concourse.tile as tile
import concourse.mybir as mybir
from concourse import bacc
from concourse import bass_utils

F32 = mybir.dt.float32
F16 = mybir.dt.float16
ALU = mybir.AluOpType
ACTF = mybir.ActivationFunctionType

# problem constants (hardcoded; kernel.py must be self-contained)
B = 32768
IN_DIM = 256
H = 512
OUT_DIM = 128
N_CORES = 8
BL = B // N_CORES          # batch per core = 4096
CHUNK = 512                # batch columns per resident chunk (1 PSUM bank/M-tile)
NCH = 2                    # resident chunks (2*4 = 8 PSUM banks)
BP = CHUNK * NCH           # batch per pass = 1024
PASSES = BL // BP          # 4
HT = H // 128              # 4 hidden tiles
IT = IN_DIM // 128         # 2 input tiles


def _pack_lhsT(wt):
    """[K, M] lhsT -> [128, (K/128)*(M/128)*128] with tile (kt, mt) at
    columns ((kt*MT)+mt)*128."""
    K, M = wt.shape
    kt, mt = K // 128, M // 128
    return np.ascontiguousarray(
        wt.reshape(kt, 128, mt, 128).transpose(1, 0, 2, 3).reshape(128, kt * mt * 128)
    )


def _pack_pp(v):
    """[H] per-hidden vector -> [128, HT] (column mt holds v[mt*128:(mt+1)*128])."""
    return np.ascontiguousarray(np.asarray(v).reshape(-1, 128).T)


def _bcast(v):
    """[H] per-hidden vector -> [128, HT*CHUNK] fp16 broadcast tile."""
    pp = _pack_pp(v)                       # [128, HT]
    return np.ascontiguousarray(
        np.repeat(pp, CHUNK, axis=1).astype(np.float16))


def _build():
    nc = bacc.Bacc("TRN2", target_bir_lowering=False, debug=False,
                   num_devices=N_CORES)

    xT_d = nc.dram_tensor("xT", [IN_DIM, BL], F16, kind="ExternalInput")
    wg1_d = nc.dram_tensor("wg1", [128, HT * HT * 128], F16, kind="ExternalInput")
    wg2_d = nc.dram_tensor("wg2", [128, HT * HT * 128], F16, kind="ExternalInput")
    m1_d = nc.dram_tensor("m1", [128, IT * HT * 128], F16, kind="ExternalInput")
    m2_d = nc.dram_tensor("m2", [128, IT * HT * 128], F16, kind="ExternalInput")
    m3_d = nc.dram_tensor("m3", [128, IT * HT * 128], F16, kind="ExternalInput")
    wf_d = nc.dram_tensor("wf", [128, HT * 128], F16, kind="ExternalInput")
    g1bc_d = nc.dram_tensor("g1bc", [128, HT * CHUNK], F16, kind="ExternalInput")
    mh_d = nc.dram_tensor("mh", [128, IT * 128], F16, kind="ExternalInput")
    bias1_d = nc.dram_tensor("bias1", [128, HT], F32, kind="ExternalInput")
    bias2_d = nc.dram_tensor("bias2", [128, HT], F32, kind="ExternalInput")
    bias3_d = nc.dram_tensor("bias3", [128, HT], F32, kind="ExternalInput")
    rbc_d = nc.dram_tensor("rbc", [128, HT * CHUNK], F16, kind="ExternalInput")
    g3bc_d = nc.dram_tensor("g3bc", [128, HT * CHUNK], F16, kind="ExternalInput")
    cy_d = nc.dram_tensor("cy", [128, 1], F32, kind="ExternalInput")
    out_d = nc.dram_tensor("out", [OUT_DIM, BL], F32, kind="ExternalOutput")

    with tile.TileContext(nc) as tc:
        with (
            tc.tile_pool(name="const", bufs=1) as cpool,
            tc.tile_pool(name="work", bufs=1) as wpool,
            tc.tile_pool(name="ppsum", bufs=1,
                         space=bass.MemorySpace.PSUM) as ppool,
        ):
            # ---- persistent weights/constants in SBUF ----
            wg1_sb = cpool.tile([128, HT * HT * 128], F16)
            wg2_sb = cpool.tile([128, HT * HT * 128], F16)
            m1_sb = cpool.tile([128, IT * HT * 128], F16)
            m2_sb = cpool.tile([128, IT * HT * 128], F16)
            m3_sb = cpool.tile([128, IT * HT * 128], F16)
            wf_sb = cpool.tile([128, HT * 128], F16)
            g1bc_sb = cpool.tile([128, HT * CHUNK], F16)
            mh_sb = cpool.tile([128, IT * 128], F16)
            bias1_sb = cpool.tile([128, HT], F32)
            bias2_sb = cpool.tile([128, HT], F32)
            bias3_sb = cpool.tile([128, HT], F32)
            rbc_sb = cpool.tile([128, HT * CHUNK], F16)
            g3bc_sb = cpool.tile([128, HT * CHUNK], F16)
            cy_sb = cpool.tile([128, 1], F32)

            # m1 in kt halves: the first P1 matmuls only wait on half
            nc.sync.dma_start(m1_sb[:, 0:HT * 128], m1_d.ap()[:, 0:HT * 128])
            nc.sync.dma_start(m1_sb[:, HT * 128:], m1_d.ap()[:, HT * 128:])
            for sb, d in [(bias1_sb, bias1_d),
                          (wg2_sb, wg2_d), (m2_sb, m2_d), (bias2_sb, bias2_d),
                          (wg1_sb, wg1_d), (rbc_sb, rbc_d), (m3_sb, m3_d),
                          (bias3_sb, bias3_d), (g3bc_sb, g3bc_d),
                          (wf_sb, wf_d), (g1bc_sb, g1bc_d),
                          (mh_sb, mh_d), (cy_sb, cy_d)]:
                nc.sync.dma_start(sb[:], d.ap())

            def sl(t, mt):
                return t[:, mt * CHUNK:(mt + 1) * CHUNK]

            def Pm(P_c, mt):
                """P is split: bank 0 (mt=0, reused by the head) is its own
                tile so the next pass's mt 1-3 matmuls only wait on the last
                tanh, not on the head-output copy."""
                lo, hi = P_c
                return lo[:] if mt == 0 else hi[:, (mt - 1) * CHUNK:mt * CHUNK]

            def mm_h(P_c, w_sb, m_c, start=False):
                """P_c += (W-tile-packed).T @ m_c, K = H (4 kt)."""
                for mt in range(HT):
                    for kt in range(HT):
                        nc.tensor.matmul(
                            Pm(P_c, mt),
                            w_sb[:, ((kt * HT) + mt) * 128:((kt * HT) + mt + 1) * 128],
                            sl(m_c, kt),
                            start=(start and kt == 0), stop=(kt == HT - 1),
                            skip_group_check=True,
                        )

            def mm_x(P_c, w_sb, x_c, start=False, mt_order=None):
                """P_c += (M-tile-packed).T @ x_c, K = IN_DIM (2 kt)."""
                for mt in (mt_order or range(HT)):
                    for kt in range(IT):
                        nc.tensor.matmul(
                            Pm(P_c, mt),
                            w_sb[:, ((kt * HT) + mt) * 128:((kt * HT) + mt + 1) * 128],
                            sl(x_c, kt),
                            start=(start and kt == 0), stop=(kt == IT - 1),
                            skip_group_check=True,
                        )

            def tanh_eval(P_c, bias_sb, c, nm):
                t = wpool.tile([128, HT * CHUNK], F16, tag=f"{nm}_{c}",
                               name=f"{nm}_{c}", bufs=1)
                for mt in range(HT):
                    nc.scalar.activation(sl(t, mt), Pm(P_c, mt), ACTF.Tanh,
                                         bias=bias_sb[:, mt:mt + 1])
                return t

            for p in range(PASSES):
                P = [(ppool.tile([128, CHUNK], F32, tag=f"Plo{c}",
                                 name=f"Plo{c}", bufs=1),
                      ppool.tile([128, (HT - 1) * CHUNK], F32, tag=f"Phi{c}",
                                 name=f"Phi{c}", bufs=1)) for c in range(NCH)]
                xs = []
                t1 = [None] * NCH
                t2 = [None] * NCH

                for c in range(NCH):
                    col0 = p * BP + c * CHUNK
                    xt = wpool.tile([128, IT * CHUNK], F16, tag=f"xt{c}",
                                    name=f"xt{c}", bufs=2)
                    # pass 0 on the scalar queue: its preamble is short and
                    # it is idle until the first tanh, so x lands ~4us in;
                    # later passes on gpsimd, off the sync/const path
                    xq = nc.scalar if p == 0 else nc.gpsimd
                    for kt in range(IT):
                        xq.dma_start(
                            xt[:, kt * CHUNK:(kt + 1) * CHUNK],
                            xT_d.ap()[kt * 128:(kt + 1) * 128,
                                      col0:col0 + CHUNK])
                    xs.append(xt)
                # mt 0 last: its bank is freed by the head copy, later than
                # the tanh that frees mt 1-3
                for c in range(NCH):
                    mm_x(P[c], m1_sb, xs[c], start=True,
                         mt_order=(1, 2, 3, 0))

                # ---- eval 1 ----
                # x-folds first: they depend only on xt, so they fill the
                # tensor queue while tanh runs
                for c in range(NCH):
                    t1[c] = tanh_eval(P[c], bias1_sb, c, "t1")
                    mm_x(P[c], m2_sb, xs[c])
                    mm_h(P[c], wg2_sb, t1[c])

                # ---- eval 2: v2 = t2 - (r/2) t1  (wg1 holds 2*Wg1) ----
                # emitted in halves so the first 8 matmuls only wait on the
                # first two tanh slices + half the vector work
                for c in range(NCH):
                    t2[c] = tanh_eval(P[c], bias2_sb, c, "t2")
                    mm_x(P[c], m3_sb, xs[c])
                    a = wpool.tile([128, HT * CHUNK], F16, tag=f"a_{c}",
                                   name=f"a_{c}", bufs=1)
                    v2 = wpool.tile([128, HT * CHUNK], F16, tag=f"v2_{c}",
                                    name=f"v2_{c}", bufs=1)
                    for h in range(2):
                        hs = slice(h * 2 * CHUNK, (h + 1) * 2 * CHUNK)
                        nc.vector.tensor_tensor(a[:, hs], t1[c][:, hs],
                                                rbc_sb[:, hs], op=ALU.mult)
                        nc.vector.tensor_tensor(v2[:, hs], t2[c][:, hs],
                                                a[:, hs], op=ALU.subtract)
                        for mt in range(HT):
                            for kt in (2 * h, 2 * h + 1):
                                nc.tensor.matmul(
                                    Pm(P[c], mt),
                                    wg1_sb[:, ((kt * HT) + mt) * 128:
                                           ((kt * HT) + mt + 1) * 128],
                                    sl(v2, kt),
                                    start=False, stop=(kt == HT - 1),
                                    skip_group_check=True)

                # ---- eval 3 + head ----
                for c in range(NCH):
                    t3 = tanh_eval(P[c], bias3_sb, c, "t3")
                    a3 = wpool.tile([128, HT * CHUNK], F16, tag=f"a3_{c}",
                                    name=f"a3_{c}", bufs=1)
                    d = wpool.tile([128, HT * CHUNK], F16, tag=f"d_{c}",
                                   name=f"d_{c}", bufs=1)
                    a1 = wpool.tile([128, HT * CHUNK], F16, tag=f"a1_{c}",
                                    name=f"a1_{c}", bufs=1)
                    hq = wpool.tile([128, HT * CHUNK], F16, tag=f"hq_{c}",
                                    name=f"hq_{c}", bufs=1)
                    # hq = t2 + (g3/g2) t3 + (g1/g2) t1  (wf holds Wf diag(g2));
                    # in halves so head matmuls start after half the chain;
                    # the x-fold goes first (depends only on xt)
                    yT = P[c][0][:OUT_DIM, 0:CHUNK]  # bank 0, dead after t3
                    for kt in range(IT):
                        nc.tensor.matmul(
                            yT, mh_sb[:, kt * 128:(kt + 1) * 128],
                            xs[c][:, kt * CHUNK:(kt + 1) * CHUNK],
                            start=(kt == 0), stop=False,
                            skip_group_check=True)
                    # final chunk runs at quarter granularity: its chain is
                    # the kernel's drain, so a 4x shorter pipe stage wins
                    nq = 4 if (p == PASSES - 1 and c == NCH - 1) else 2
                    st = HT // nq
                    for h in range(nq):
                        hs = slice(h * st * CHUNK, (h + 1) * st * CHUNK)
                        nc.vector.tensor_tensor(a3[:, hs], t3[:, hs],
                                                g3bc_sb[:, hs], op=ALU.mult)
                        nc.vector.tensor_tensor(d[:, hs], t2[c][:, hs],
                                                a3[:, hs], op=ALU.add)
                        nc.vector.tensor_tensor(a1[:, hs], t1[c][:, hs],
                                                g1bc_sb[:, hs], op=ALU.mult)
                        nc.vector.tensor_tensor(hq[:, hs], d[:, hs],
                                                a1[:, hs], op=ALU.add)
                        for kt in range(h * st, (h + 1) * st):
                            nc.tensor.matmul(
                                yT, wf_sb[:, kt * 128:(kt + 1) * 128],
                                sl(hq, kt), start=False,
                                stop=(kt == HT - 1),
                                skip_group_check=True)
                    ob = wpool.tile([OUT_DIM, CHUNK], F32, tag=f"ob{c}",
                                    name=f"ob{c}", bufs=2)
                    # vector, not scalar: scalar is mid-tanh for the other
                    # chunk exactly when this lands, and the next pass's P1
                    # waits on this read (bank-0 tag rotation)
                    nc.vector.tensor_scalar(ob[:], yT, cy_sb[:, 0:1], None,
                                            op0=ALU.add)
                    col0 = p * BP + c * CHUNK
                    nc.sync.dma_start(out_d.ap()[:, col0:col0 + CHUNK],
                                      ob[:])

    nc.compile()
    return nc


_CACHED = None
RUN_KWARGS = {}
LAST_RESULT = None


def _get_nc():
    global _CACHED
    if _CACHED is None:
        _CACHED = _build()
    return _CACHED


def kernel(x, Wx, bx, W, U, b, tau, Wf, bf):
    x = np.asarray(x, np.float32)
    Wx = np.asarray(Wx, np.float64)
    bx = np.asarray(bx, np.float64)
    W = np.asarray(W, np.float64)
    U = np.asarray(U, np.float64)
    b = np.asarray(b, np.float64)
    tau = np.asarray(tau, np.float64)
    Wf = np.asarray(Wf, np.float64)
    bf = np.asarray(bf, np.float64)

    # ---- ETDRK3 (Cox-Matthews) coefficients, dt = 1, L = -1/tau ----
    z = -1.0 / tau
    z2 = 0.5 * z
    E2 = np.exp(z2)
    E = np.exp(z)

    def phi1(v): return np.expm1(v) / v
    def phi2(v): return (np.expm1(v) - v) / v**2
    def phi3(v): return (np.expm1(v) - v - v * v / 2) / v**3

    gam2 = 0.5 * phi1(z2) / tau
    gam1 = phi1(z) / tau
    g1 = (4 * phi3(z) - 3 * phi2(z) + phi1(z)) / tau
    g2 = (4 * phi2(z) - 8 * phi3(z)) / tau
    g3 = (4 * phi3(z) - phi2(z)) / tau

    M1 = (W + U) @ Wx
    c1 = (W + U) @ bx
    M2 = (W * (E2 - 1.0)[None, :]) @ Wx
    c2 = W @ ((E2 - 1.0) * bx)
    M3 = (W * (E - E2)[None, :]) @ Wx
    c3 = W @ ((E - E2) * bx)
    Mh = (Wf * E[None, :]) @ Wx
    cy = Wf @ (E * bx) + bf
    Wg2 = W * gam2[None, :]
    Wg1 = W * (2.0 * gam1)[None, :]       # the "2 t2" factor folded in
    Wfd = Wf * g2[None, :]                # head group carries g2
    r = 0.5 * (gam1 + gam2) / gam1        # so v2 = t2 - r t1

    bias1 = b + c1
    bias2 = bias1 + c2
    bias3 = bias2 + c3

    weights = {
        "wg1": _pack_lhsT(Wg1.T.astype(np.float16)),
        "wg2": _pack_lhsT(Wg2.T.astype(np.float16)),
        "m1": _pack_lhsT(M1.T.astype(np.float16)),
        "m2": _pack_lhsT(M2.T.astype(np.float16)),
        "m3": _pack_lhsT(M3.T.astype(np.float16)),
        "wf": _pack_lhsT(Wfd.T.astype(np.float16)),
        "g1bc": _bcast(g1 / g2),
        "mh": _pack_lhsT(Mh.T.astype(np.float16)),
        "bias1": _pack_pp(bias1.astype(np.float32)),
        "bias2": _pack_pp(bias2.astype(np.float32)),
        "bias3": _pack_pp(bias3.astype(np.float32)),
        "rbc": _bcast(r),
        "g3bc": _bcast(g3 / g2),
        "cy": np.ascontiguousarray(cy.astype(np.float32).reshape(128, 1)),
    }

    nc = _get_nc()
    in_maps = []
    for c in range(N_CORES):
        m = dict(weights)
        m["xT"] = np.ascontiguousarray(
            x[c * BL:(c + 1) * BL].T.astype(np.float16))
        in_maps.append(m)
    res = bass_utils.run_bass_kernel_spmd(nc, in_maps,
                                          core_ids=list(range(N_CORES)),
                                          **RUN_KWARGS)
    global LAST_RESULT
    LAST_RESULT = res
    return np.ascontiguousarray(
        np.concatenate([res.results[c]["out"].T for c in range(N_CORES)],
                       axis=0))
